# revision 1
# baseline (speedup 1.0000x reference)
"""Self-contained Trainium2 (Bass/Tile) kernel for nn_CQAttention.

kernel(**inputs) takes FULL inputs (B=64) and returns the FULL output
[64, 2048, 512] (= concat[C, A, C*A, C*Bm]). Internally shards batch across
8 NeuronCores (data parallel, 8 batches/core) and runs a Bass/Tile program
via concourse.bass_utils.run_bass_kernel_spmd.

Math (per batch; bias is a constant shift so it cancels in both softmaxes):
  s2[c,q] = sum_d C[c,d]*w4mul[d]*Q[q,d];  s0[c] = C@w4c;  s1[q] = Q@w4q
  S  = s2 + s0 + s1
  S1 = softmax_q(S);  S2 = softmax_c(S)
  A  = S1 @ Q
  Bm = S1 @ (S2^T @ C)      # associativity: (S1 S2^T) C == S1 (S2^T C)
  out = [C, A, C*A, C*Bm]
"""
import sys
import numpy as np

for _p in ("/opt/trn_rl_repo",):
    if _p not in sys.path:
        sys.path.insert(0, _p)

import concourse.bass as bass
import concourse.mybir as mybir
import concourse.tile as tile
from concourse import bacc
from concourse.masks import make_identity
from concourse.bass_utils import run_bass_kernel_spmd
from contextlib import ExitStack

F32 = mybir.dt.float32
F32R = mybir.dt.float32r
BF16 = mybir.dt.bfloat16
AF = mybir.ActivationFunctionType
AX = mybir.AxisListType

N_CORES = 8
EPS_BUFS, MISC_BUFS, MAB_BUFS = 4, 2, 2
B, CL, QL, D = 64, 2048, 512, 128
NB = B // N_CORES  # batches per core


def _build_body(nc, tc, ctx, nb, cl, ql, d, C_d, Q_d, w4c_d, w4q_d, w4m_d, OUT_d):
    NT = cl // 128
    NQ = ql // 128
    NCC = cl // 512
    NG = NT // 4   # c-tile groups of 4

    consts = ctx.enter_context(tc.tile_pool(name="consts", bufs=1))
    ident = consts.tile([128, 128], F32)
    make_identity(nc, ident)
    w4c_sb = consts.tile([d, 1], F32)
    nc.sync.dma_start(w4c_sb, w4c_d)
    w4q_sb = consts.tile([d, 1], F32)
    nc.sync.dma_start(w4q_sb, w4q_d)
    w4m_sb = consts.tile([d, 1], F32)
    nc.sync.dma_start(w4m_sb, w4m_d)
    # fp32r matmul operands must be produced as float32r (rounded)
    w4c_r = consts.tile([d, 1], F32R)
    nc.vector.tensor_copy(w4c_r, w4c_sb)
    w4q_r = consts.tile([d, 1], F32R)
    nc.vector.tensor_copy(w4q_r, w4q_sb)
    # f32r ones row for the rank-2 stacked tiles (memset can't emit f32r)
    ones_f = consts.tile([1, cl], F32)
    nc.gpsimd.memset(ones_f, 1.0)
    ones_r = consts.tile([1, cl], F32R)
    nc.vector.tensor_copy(ones_r, ones_f)

    ld = ctx.enter_context(tc.tile_pool(name="ld", bufs=2))
    ctp = ctx.enter_context(tc.tile_pool(name="ctp", bufs=2))
    small = ctx.enter_context(tc.tile_pool(name="small", bufs=2))
    epool = ctx.enter_context(tc.tile_pool(name="epool", bufs=2))
    stat = ctx.enter_context(tc.tile_pool(name="stat", bufs=2))
    outp = ctx.enter_context(tc.tile_pool(name="outp", bufs=3))
    csp = ctx.enter_context(tc.tile_pool(name="csp", bufs=2))
    stp = ctx.enter_context(tc.tile_pool(name="stp", bufs=2))

    e_ps = ctx.enter_context(tc.tile_pool(name="e_ps", bufs=EPS_BUFS, space="PSUM"))
    misc_ps = ctx.enter_context(tc.tile_pool(name="misc_ps", bufs=MISC_BUFS, space="PSUM"))
    mab_ps = ctx.enter_context(tc.tile_pool(name="mab_ps", bufs=MAB_BUFS, space="PSUM"))

    def bcast(ap2d, n):
        # [128, k] -> [128, k, n] with a step-0 inner dim
        return ap2d.unsqueeze(2).broadcast_to((128, ap2d.shape[1], n))

    for b in range(nb):
        # ---- loads ----
        c_nat = ld.tile([128, NT, d], F32, tag="c_nat")
        nc.sync.dma_start(c_nat, C_d[b].rearrange("(t p) d -> p t d", p=128))
        q_nat = ld.tile([128, NQ, d], F32, tag="q_nat")
        nc.sync.dma_start(q_nat, Q_d[b].rearrange("(t p) d -> p t d", p=128))

        # ---- PE transposes, psum-grouped by 4 so each drain is one big copy
        ct_t = ctp.tile([128, NT, d], F32R, tag="ct")    # [d, t, c]
        for g in range(NG):
            tp = misc_ps.tile([128, 4, 128], F32, tag="m", name="tp")
            for i in range(4):
                nc.tensor.transpose(tp[:, i, :], c_nat[:, g * 4 + i, :], ident)
            nc.vector.tensor_copy(ct_t[:, g * 4:(g + 1) * 4, :], tp)
        qt_t = small.tile([128, NQ, d], F32R, tag="qt")  # [d, qt, q]
        tpq = misc_ps.tile([128, 4, 128], F32, tag="m", name="tpq")
        for i in range(NQ):
            nc.tensor.transpose(tpq[:, i, :], q_nat[:, i, :], ident)
        nc.vector.tensor_copy(qt_t, tpq)

        ct_flat = ct_t.rearrange("p a b -> p (a b)")     # [d, cl]
        qt_flat = qt_t.rearrange("p a b -> p (a b)")     # [d, ql]

        qwt = small.tile([128, ql], F32R, tag="qwt")     # QT * w4mul
        nc.vector.tensor_scalar_mul(qwt, qt_flat, w4m_sb)

        # bf16 copies for the averaging matmuls (gpsimd DMA does the cast)
        c_bf = small.tile([128, NT, d], BF16, tag="c_bf")
        nc.gpsimd.dma_start(c_bf, c_nat)
        q_bf = small.tile([128, NQ, d], BF16, tag="q_bf")
        nc.gpsimd.dma_start(q_bf, q_nat)

        # ---- rank-2 stacked tiles: st_c rows [s0; ones], st_q rows [ones; s1]
        st_c = stp.tile([2, cl], F32R, tag="stc")
        nc.sync.dma_start(st_c[1:2, :], ones_r)
        for j in range(NCC):
            sp = misc_ps.tile([1, 512], F32, tag="m", name="sp")
            nc.tensor.matmul(sp, w4c_r,
                             ct_flat[:, j * 512:(j + 1) * 512])
            nc.vector.tensor_copy(st_c[0:1, j * 512:(j + 1) * 512], sp)
        st_q = stp.tile([2, ql], F32R, tag="stq")
        nc.vector.tensor_copy(st_q[0:1, :], ones_r[:, 0:ql])
        sq = misc_ps.tile([1, 512], F32, tag="m", name="sq")
        nc.tensor.matmul(sq, w4q_r, qt_flat)
        # DVE cannot write at base partition 1 and DMA cannot read PSUM:
        # bounce s1 through an SBUF row then SBUF->SBUF DMA into row 1
        s1_tmp = small.tile([1, ql], F32R, tag="s1_tmp")
        nc.vector.tensor_copy(s1_tmp, sq)
        nc.sync.dma_start(st_q[1:2, :], s1_tmp)

        # ---- E_c + E_q passes interleaved (better PE<->ACT pipelining) ----
        ec = epool.tile([128, NT, ql], BF16, tag="ec")
        d1 = stat.tile([128, NT], F32, tag="d1")
        eq = epool.tile([128, NQ, cl], BF16, tag="eq")
        qacc = stat.tile([128, NQ, NCC], F32, tag="qacc")
        for t in range(NT):
            ep = e_ps.tile([128, ql], F32, tag="e", name="ep")
            nc.tensor.matmul(ep, ct_t[:, t, :], qwt,
                             start=True, stop=False)
            nc.tensor.matmul(ep, st_c[:, t * 128:(t + 1) * 128],
                             st_q, start=False, stop=True)
            nc.scalar.activation(ec[:, t, :], ep, AF.Exp, accum_out=d1[:, t:t + 1])
            qi, j = t // NCC, t % NCC
            ep2 = e_ps.tile([128, 512], F32, tag="e", name="ep2")
            nc.tensor.matmul(ep2, qwt[:, qi * 128:(qi + 1) * 128],
                             ct_flat[:, j * 512:(j + 1) * 512],
                             start=True, stop=False)
            nc.tensor.matmul(ep2, st_q[:, qi * 128:(qi + 1) * 128],
                             st_c[:, j * 512:(j + 1) * 512],
                             start=False, stop=True)
            nc.scalar.activation(eq[:, qi, j * 512:(j + 1) * 512], ep2, AF.Exp,
                                 accum_out=qacc[:, qi, j:j + 1])
        rrow = stat.tile([128, NT], F32, tag="rrow")
        nc.vector.reciprocal(rrow, d1)
        colsum = stat.tile([128, NQ], F32, tag="colsum")
        nc.vector.reduce_sum(colsum, qacc, axis=AX.X)
        rcol = stat.tile([128, NQ], F32, tag="rcol")
        nc.vector.reciprocal(rcol, colsum)

        # ---- M' = (E_c^T C) * rcol -> bf16 [q, d]; all 4 q-chunks in 1 bank
        m_bf = small.tile([128, NQ, d], BF16, tag="m_bf")
        mp = mab_ps.tile([128, NQ, 128], F32, tag="mab", name="mp")
        for qi in range(NQ):
            for t in range(NT):
                nc.tensor.matmul(mp[:, qi, :], ec[:, t, qi * 128:(qi + 1) * 128],
                                 c_bf[:, t, :], start=(t == 0), stop=(t == NT - 1))
        nc.vector.tensor_mul(m_bf, mp, bcast(rcol, 128))

        # ---- A_raw, Bm_raw and output, grouped by 4 c-tiles ----
        out_r = OUT_d[b].rearrange("(t p) n -> p t n", p=128)
        for g in range(NG):
            apg = mab_ps.tile([128, 4, 128], F32, tag="mab", name="apg")
            bpg = mab_ps.tile([128, 4, 128], F32, tag="mab", name="bpg")
            for i in range(4):
                t = g * 4 + i
                for qi in range(NQ):
                    lhs = eq[:, qi, t * 128:(t + 1) * 128]
                    nc.tensor.matmul(apg[:, i, :], lhs, q_bf[:, qi, :],
                                     start=(qi == 0), stop=(qi == NQ - 1))
                    nc.tensor.matmul(bpg[:, i, :], lhs, m_bf[:, qi, :],
                                     start=(qi == 0), stop=(qi == NQ - 1))
            rb = bcast(rrow[:, g * 4:(g + 1) * 4], 128)
            cs4 = csp.tile([128, 4, 128], F32, tag="cs")
            nc.vector.tensor_mul(cs4, c_nat[:, g * 4:(g + 1) * 4, :], rb)
            ob = outp.tile([128, 4, 384], F32, tag="ob")
            nc.vector.tensor_mul(ob[:, :, 0:128], apg, rb)
            nc.vector.tensor_mul(ob[:, :, 128:256], apg, cs4)
            nc.vector.tensor_mul(ob[:, :, 256:384], bpg, cs4)
            nc.gpsimd.dma_start(out_r[:, g * 4:(g + 1) * 4, 128:512], ob)
            nc.sync.dma_start(out_r[:, g * 4:(g + 1) * 4, 0:128],
                              c_nat[:, g * 4:(g + 1) * 4, :])


def build_program(nb=NB):
    nc = bacc.Bacc("TRN2", target_bir_lowering=False, debug=False,
                   num_devices=N_CORES)
    C_d = nc.dram_tensor("C", [nb, CL, D], F32, kind="ExternalInput").ap()
    Q_d = nc.dram_tensor("Q", [nb, QL, D], F32, kind="ExternalInput").ap()
    w4c_d = nc.dram_tensor("w4c", [D, 1], F32, kind="ExternalInput").ap()
    w4q_d = nc.dram_tensor("w4q", [D, 1], F32, kind="ExternalInput").ap()
    w4m_d = nc.dram_tensor("w4mul", [D, 1], F32, kind="ExternalInput").ap()
    OUT_d = nc.dram_tensor("OUT", [nb, CL, 4 * D], F32, kind="ExternalOutput").ap()
    with ExitStack() as ctx:
        tc = ctx.enter_context(tile.TileContext(nc))
        _build_body(nc, tc, ctx, nb, CL, QL, D,
                    C_d, Q_d, w4c_d, w4q_d, w4m_d, OUT_d)
    nc.compile()
    return nc


_PROGRAM_CACHE = {}


def _get_program(nb=NB):
    if nb not in _PROGRAM_CACHE:
        _PROGRAM_CACHE[nb] = build_program(nb)
    return _PROGRAM_CACHE[nb]


def _numpy_fallback(C, Q, c_mask, q_mask, w4c, w4q, w4mul, bias):
    """Exact reference math in numpy (used only if masks are not all-ones)."""
    NEG_INF = -1e30
    out = np.empty((C.shape[0], C.shape[1], 4 * C.shape[2]), np.float32)
    for b in range(C.shape[0]):
        Cb = C[b].astype(np.float64)
        Qb = Q[b].astype(np.float64)
        S = (Cb @ w4c.reshape(-1, 1) + (Qb @ w4q.reshape(-1, 1)).T
             + (Cb * w4mul.reshape(1, -1)) @ Qb.T + float(np.asarray(bias).reshape(-1)[0]))
        qm = q_mask[b].reshape(1, -1)
        cm = c_mask[b].reshape(-1, 1)
        S1l = S * qm + NEG_INF * (1.0 - qm)
        S2l = S * cm + NEG_INF * (1.0 - cm)
        S1 = np.exp(S1l - S1l.max(1, keepdims=True))
        S1 /= S1.sum(1, keepdims=True)
        S2 = np.exp(S2l - S2l.max(0, keepdims=True))
        S2 /= S2.sum(0, keepdims=True)
        A = S1 @ Qb
        Bm = S1 @ (S2.T @ Cb)
        out[b] = np.concatenate([Cb, A, Cb * A, Cb * Bm], axis=1).astype(np.float32)
    return out


def kernel(C, Q, c_mask, q_mask, w4c, w4q, w4mul, bias):
    C = np.ascontiguousarray(np.asarray(C), dtype=np.float32)
    Q = np.ascontiguousarray(np.asarray(Q), dtype=np.float32)
    c_mask = np.asarray(c_mask)
    q_mask = np.asarray(q_mask)
    w4c = np.asarray(w4c, dtype=np.float32)
    w4q = np.asarray(w4q, dtype=np.float32)
    w4mul = np.asarray(w4mul, dtype=np.float32)

    if not (np.all(c_mask == 1.0) and np.all(q_mask == 1.0)):
        return _numpy_fallback(C, Q, c_mask, q_mask, w4c, w4q, w4mul, bias)

    nc = _get_program(NB)
    w4c_r = np.ascontiguousarray(w4c.reshape(D, 1))
    w4q_r = np.ascontiguousarray(w4q.reshape(D, 1))
    w4m_r = np.ascontiguousarray(w4mul.reshape(D, 1))
    in_maps = []
    for c in range(N_CORES):
        sl = slice(c * NB, (c + 1) * NB)
        in_maps.append({
            "C": np.ascontiguousarray(C[sl]),
            "Q": np.ascontiguousarray(Q[sl]),
            "w4c": w4c_r,
            "w4q": w4q_r,
            "w4mul": w4m_r,
        })
    res = run_bass_kernel_spmd(nc, in_maps, core_ids=list(range(N_CORES)))
    out = np.concatenate([res.results[c]["OUT"] for c in range(N_CORES)], axis=0)
    return out

